# revision 1
# baseline (speedup 1.0000x reference)
"""Grouped-query attention kernel for 8 Trainium2 NeuronCores.

Problem (hardcoded): x [2, 512, 16, 16, 16] f32, Wq/Wk/Wv/Wo [512, 512],
biases [512]. G=4 heads of dim 128, N=4096 tokens. out = x + Wo@attn.

Sharding: one (batch, group) pair per core -> 8 cores, no cross-core
communication. Each core computes its group's Q/K/V projections, the
full 4096x4096 attention for its (b, g), and a partial output
projection Wo[:, g_cols] @ O_g -> [512, 4096]. Host sums the 4 partials
per batch and adds the residual + bo.

Device-side layout (per core):
  - xf (x[b] as [512, 4096]) bf16, 4 chunks of [128, 4096] in SBUF
  - Q, K: [128(gs), 4096] bf16; V^T: [128(keys-chunk), 32*128] bf16
  - per query tile (512 wide): S^T chunk = K_chunk^T Q_tile -> PSUM,
    exp on ScalarE (scale folded in) -> E^T bf16, then accumulate
    O += V^T_chunk^T E^T and denom += ones^T E^T on TensorE.
    Normalize via reciprocal + ones-broadcast matmul, then Wo partial.
"""

import os
import numpy as np
import ml_dtypes

B, C, N, G = 2, 512, 4096, 4
GS = C // G          # 128 head dim
SCALE = GS ** -0.5
QT = 512             # query tile width
NQT = N // QT        # 8 query tiles
NKC = N // 128       # 32 key chunks
NCC = C // 128       # 4 contraction chunks for projections
NMC = C // 128       # 4 output-channel chunks

_compiled_nc = None
LAST_RESULT = None


def _build():
    from contextlib import ExitStack
    import concourse.mybir as mybir
    import concourse.tile as tile
    from concourse import bacc

    dt = mybir.dt
    f32 = dt.float32
    bf16 = dt.bfloat16
    Exp = mybir.ActivationFunctionType.Exp

    nc = bacc.Bacc("TRN2", target_bir_lowering=False, debug=False, num_devices=8)

    xb = nc.dram_tensor("xb", [C, N], bf16, kind="ExternalInput")
    wqT = nc.dram_tensor("wqT", [C, GS], bf16, kind="ExternalInput")
    wkT = nc.dram_tensor("wkT", [C, GS], bf16, kind="ExternalInput")
    wvT = nc.dram_tensor("wvT", [C, GS], bf16, kind="ExternalInput")
    woT = nc.dram_tensor("woT", [GS, C], bf16, kind="ExternalInput")
    bq = nc.dram_tensor("bq", [GS, 1], f32, kind="ExternalInput")
    bk = nc.dram_tensor("bk", [GS, 1], f32, kind="ExternalInput")
    bvb = nc.dram_tensor("bvb", [128, GS], f32, kind="ExternalInput")
    outp = nc.dram_tensor("outp", [C, N], f32, kind="ExternalOutput")

    KG = 2                    # key chunks per exp group
    GW = KG * 128             # exp group width in keys
    NGR = N // GW             # 16 groups per query tile

    with tile.TileContext(nc) as tc, ExitStack() as ctx:
        persist = ctx.enter_context(tc.tile_pool(name="persist", bufs=1))
        epool = ctx.enter_context(tc.tile_pool(name="epool", bufs=4))
        # bufs=4: tail chains of consecutive q-tiles otherwise couple
        # through slot reuse and cascade-delay each other
        spool = ctx.enter_context(tc.tile_pool(name="spool", bufs=4))
        # PSUM budget (8 banks): psS 2x[128,1024]=4, psO 2x[128,512]=2,
        # psD 1, psP 1.
        psS = ctx.enter_context(tc.tile_pool(name="psS", bufs=2, space="PSUM"))
        psO = ctx.enter_context(tc.tile_pool(name="psO", bufs=2, space="PSUM"))
        psD = ctx.enter_context(tc.tile_pool(name="psD", bufs=1, space="PSUM"))
        psP = ctx.enter_context(tc.tile_pool(name="psP", bufs=1, space="PSUM"))

        def load(shape, dtype, dram_ap, tag):
            t = persist.tile(shape, dtype, tag=tag)
            nc.sync.dma_start(t[:], dram_ap)
            return t

        # Weights first: they gate the first projection matmuls.
        wq = [load([128, GS], bf16, wqT[cc * 128:(cc + 1) * 128, :], f"wq{cc}")
              for cc in range(NCC)]
        wk = [load([128, GS], bf16, wkT[cc * 128:(cc + 1) * 128, :], f"wk{cc}")
              for cc in range(NCC)]
        wv = [load([128, GS], bf16, wvT[cc * 128:(cc + 1) * 128, :], f"wv{cc}")
              for cc in range(NCC)]
        wo_sb = load([GS, C], bf16, woT[:, :], "wo")
        bq_sb = load([GS, 1], f32, bq[:, :], "bq")
        bk_sb = load([GS, 1], f32, bk[:, :], "bk")
        bvb_sb = load([128, GS], f32, bvb[:, :], "bvb")

        xf = [[None] * NQT for _ in range(NCC)]

        def load_xf(nt):
            for cc in range(NCC):
                xf[cc][nt] = load(
                    [128, QT], bf16,
                    xb[cc * 128:(cc + 1) * 128, nt * QT:(nt + 1) * QT],
                    f"xf{cc}_{nt}")

        ones_k = persist.tile([128, 1], bf16, tag="ones_k")
        nc.vector.memset(ones_k[:], 1.0)
        ones_1 = persist.tile([1, 128], bf16, tag="ones_1")
        nc.vector.memset(ones_1[:], 1.0)

        q_sb = persist.tile([GS, N], bf16, tag="q_sb")
        k_sb = persist.tile([GS, N], bf16, tag="k_sb")
        vt_sb = persist.tile([128, N], bf16, tag="vt_sb")

        # Projections, emitted per column block (nt) with its xf loads
        # inline so compute starts after ~4 DMAs and overlaps the rest.
        # Q/K: [gs, N] = W_g @ xf (+ bias per partition);
        # V^T: [keys, gs] per 128-key chunk = xf_chunk^T @ Wv_g^T.
        load_xf(0)
        for nt in range(NQT):
            if nt + 1 < NQT:
                load_xf(nt + 1)
            nsl = slice(nt * QT, (nt + 1) * QT)
            for w_t, b_t, dst in ((wq, bq_sb, q_sb), (wk, bk_sb, k_sb)):
                ps = psO.tile([128, QT], f32, tag="po")
                for cc in range(NCC):
                    nc.tensor.matmul(ps[:], w_t[cc][:], xf[cc][nt][:],
                                     start=(cc == 0), stop=(cc == NCC - 1))
                nc.vector.tensor_scalar_add(dst[:, nsl], ps[:], b_t[:])
            for kc in range(nt * QT // 128, (nt + 1) * QT // 128):
                ksl = slice(kc * 128, (kc + 1) * 128)
                off = kc * 128 - nt * QT
                ps = psS.tile([128, GS], f32, tag="ps")
                for cc in range(NCC):
                    nc.tensor.matmul(ps[:], xf[cc][nt][:, off:off + 128],
                                     wv[cc][:],
                                     start=(cc == 0), stop=(cc == NCC - 1))
                nc.vector.tensor_add(vt_sb[:, ksl], ps[:], bvb_sb[:])

        # Attention, software-pipelined per query tile.
        # PE order per group g: S(g+1) matmuls, then O/D(g) — so PE stays
        # dense while ScalarE runs exp(g). Wide exp over KG key chunks.
        def emit_S(qt, g):
            qsl = slice(qt * QT, (qt + 1) * QT)
            ps = psS.tile([128, GW // 128 * QT], f32, tag="ps")
            for j in range(KG):
                kc = g * KG + j
                ksl = slice(kc * 128, (kc + 1) * 128)
                nc.tensor.matmul(ps[:, j * QT:(j + 1) * QT],
                                 k_sb[:, ksl], q_sb[:, qsl],
                                 start=True, stop=True)
            return ps

        tails = []

        def emit_tail(qt, po, pd):
            state = {}

            def tail_pre():
                # free the pd bank + start the reciprocal chain early
                den_sb = spool.tile([1, QT], bf16, tag="den")
                nc.vector.tensor_copy(den_sb[:], pd[:])
                pb = psP.tile([128, QT], f32, tag="pp")
                nc.tensor.matmul(pb[:], ones_1[:], den_sb[:],
                                 start=True, stop=True)
                binv = spool.tile([128, QT], f32, tag="binv")
                nc.vector.reciprocal(binv[:], pb[:])
                state["binv"] = binv

            def tail_main():
                qsl = slice(qt * QT, (qt + 1) * QT)
                o_sb = spool.tile([128, QT], bf16, tag="osb")
                nc.vector.tensor_mul(o_sb[:], po[:], state["binv"][:])
                for mc in range(NMC):
                    msl = slice(mc * 128, (mc + 1) * 128)
                    pp = psP.tile([128, QT], f32, tag="pp")
                    nc.tensor.matmul(pp[:], wo_sb[:, msl], o_sb[:],
                                     start=True, stop=True)
                    st = spool.tile([128, QT], f32, tag="st")
                    nc.vector.tensor_copy(st[:], pp[:])
                    nc.sync.dma_start(outp[msl, qsl], st[:])
            return tail_pre, tail_main

        for qt in range(NQT):
            po = psO.tile([128, QT], f32, tag="po")
            s_cur = emit_S(qt, 0)
            if tails:
                tails[-1][0]()         # prev epilogue: den copy + bcast + recip
            pd = psD.tile([1, QT], f32, tag="pd")
            for g in range(NGR):
                # rest of prev epilogue two groups in: by now its DVE
                # chain is done, so the Wo matmuls don't stall PE
                if g == 2 and tails:
                    tails.pop()[1]()
                s_next = emit_S(qt, g + 1) if g + 1 < NGR else None
                e = epool.tile([128, GW // 128 * QT], bf16, tag="e")
                nc.scalar.activation(e[:], s_cur[:], Exp, scale=SCALE)
                # same-psum-bank matmuls back-to-back: [O,O] then [D,D]
                # (interleaving accumulating matmuls across banks measured
                # ~1.5x slower per matmul)
                for j in range(KG):
                    kc = g * KG + j
                    ksl = slice(kc * 128, (kc + 1) * 128)
                    esl = slice(j * QT, (j + 1) * QT)
                    nc.tensor.matmul(po[:], vt_sb[:, ksl], e[:, esl],
                                     start=(kc == 0), stop=(kc == NKC - 1))
                for j in range(KG):
                    kc = g * KG + j
                    esl = slice(j * QT, (j + 1) * QT)
                    nc.tensor.matmul(pd[:], ones_k[:], e[:, esl],
                                     start=(kc == 0), stop=(kc == NKC - 1))
                s_cur = s_next
            tails.append(emit_tail(qt, po, pd))
        tp, tm = tails.pop()
        tp()
        tm()

    nc.compile()
    return nc


def _get_compiled():
    global _compiled_nc
    if _compiled_nc is None:
        _compiled_nc = _build()
    return _compiled_nc


def _ensure_ntff_hook():
    """Best-effort: register the axon NTFF profile hook so trace=True
    yields exec_time_ns. The image's antenv lacks axon_hooks; shim it."""
    import sys, types
    try:
        from antenv.axon_hooks import get_axon_ntff_profile_hook  # noqa: F401
        return
    except ImportError:
        pass
    try:
        mod = types.ModuleType("antenv.axon_hooks")
        _hook = [None]
        mod.set_axon_ntff_profile_hook = lambda h: _hook.__setitem__(0, h)
        mod.get_axon_ntff_profile_hook = lambda: _hook[0]
        sys.modules["antenv.axon_hooks"] = mod
        import antenv
        antenv.axon_hooks = mod
        from trn_agent_boot.trn_boot import _ntff_profile_via_ctypes
        mod.set_axon_ntff_profile_hook(
            _ntff_profile_via_ctypes("/opt/axon/libaxon_pjrt.so"))
    except Exception:
        pass


def kernel(x, Wq, bq, Wk, bk, Wv, bv, Wo, bo):
    global LAST_RESULT
    from concourse.bass_utils import run_bass_kernel_spmd

    nc = _get_compiled()
    bf = ml_dtypes.bfloat16
    x = np.asarray(x, dtype=np.float32)
    b, c, d, h, w = x.shape
    n = d * h * w
    xf = x.reshape(b, c, n)
    Wq = np.asarray(Wq, np.float32)
    Wk = np.asarray(Wk, np.float32)
    Wv = np.asarray(Wv, np.float32)
    Wo = np.asarray(Wo, np.float32)
    bq = np.asarray(bq, np.float32)
    bk = np.asarray(bk, np.float32)
    bv = np.asarray(bv, np.float32)
    bo = np.asarray(bo, np.float32)

    in_maps = []
    for core in range(8):
        bb, g = divmod(core, G)
        gsl = slice(g * GS, (g + 1) * GS)
        in_maps.append({
            "xb": np.ascontiguousarray(xf[bb]).astype(bf),
            "wqT": np.ascontiguousarray(Wq[gsl, :].T).astype(bf),
            "wkT": np.ascontiguousarray(Wk[gsl, :].T).astype(bf),
            "wvT": np.ascontiguousarray(Wv[gsl, :].T).astype(bf),
            "woT": np.ascontiguousarray(Wo[:, gsl].T).astype(bf),
            "bq": bq[gsl].reshape(GS, 1).copy(),
            "bk": bk[gsl].reshape(GS, 1).copy(),
            "bvb": np.ascontiguousarray(np.broadcast_to(bv[gsl], (128, GS))),
        })

    trace = bool(os.environ.get("BASS_TRACE"))
    if trace:
        _ensure_ntff_hook()
    LAST_RESULT = run_bass_kernel_spmd(
        nc, in_maps, core_ids=list(range(8)), trace=trace)
    outs = LAST_RESULT.results

    out = np.empty((b, c, n), np.float32)
    for bb in range(b):
        acc = xf[bb] + bo[:, None]
        for g in range(G):
            acc = acc + outs[bb * G + g]["outp"]
        out[bb] = acc
    return out.reshape(b, c, d, h, w)



# revision 6
# speedup vs baseline: 1.3104x; 1.3104x over previous
"""Grouped-query attention kernel for 8 Trainium2 NeuronCores.

Problem (hardcoded): x [2, 512, 16, 16, 16] f32, Wq/Wk/Wv/Wo [512, 512],
biases [512]. G=4 heads of dim 128, N=4096 tokens. out = x + Wo@attn.

Sharding: one (batch, group) pair per core -> 8 cores, no cross-core
communication. Each core computes its group's Q/K/V projections, the
full 4096x4096 attention for its (b, g), and a partial output
projection Wo[:, g_cols] @ O_g -> [512, 4096]. Host sums the 4 partials
per batch and adds the residual + bo.

fp8 strategy: weights are pre-scaled x16 so their N(0, 1/sqrt(C))
entries land in fp8e4's normal range; x, Wq/Wk/Wv, V^T and the exp'd
scores all live in fp8e4 so every big matmul runs in DoubleRow perf
mode (two 128-deep contraction chunks per PE pass). The x16 scales are
unwound via the exp scale (/256) and Wo (/16 on host). exp also folds
a -2 bias (uniform across keys, cancels in softmax) to keep e^s < 240.

Device-side layout (per core):
  - xf (x[b] as [128, 4, 4096]) fp8, per q-tile chunks in SBUF
  - Q, K: [128(gs), 4096] bf16; V^T: [128, 32, 128] fp8
  - per query tile (512 wide), per pair of 256-key exp groups:
    S^T = K_chunk^T Q_tile -> PSUM (bf16 matmuls), exp on ScalarE
    (scale+bias folded) -> E fp8 [128, 2, 512], then DoubleRow
    accumulate O += V^T E and denom += ones^T E on TensorE, batching
    same-bank accumulating matmuls back-to-back ([O,O] then [D,D]).
    Normalize via reciprocal + ones-broadcast matmul, then Wo partial.
"""

import os
import numpy as np
import ml_dtypes

B, C, N, G = 2, 512, 4096, 4
GS = C // G          # 128 head dim
SCALE = GS ** -0.5
QT = 512             # query tile width
NQT = N // QT        # 8 query tiles
NKC = N // 128       # 32 key chunks
NCC = C // 128       # 4 contraction chunks for projections
NMC = C // 128       # 4 output-channel chunks
KG = 2               # key chunks per exp group (= DoubleRow pair)
NGR = NKC // KG      # 16 groups per query tile
WS = 16.0            # fp8 weight pre-scale
EXP_BIAS = -2.0      # uniform logit shift, cancels in softmax

_compiled_nc = None
LAST_RESULT = None


def _build():
    from contextlib import ExitStack
    import concourse.mybir as mybir
    import concourse.tile as tile
    from concourse import bacc

    dt = mybir.dt
    f32 = dt.float32
    bf16 = dt.bfloat16
    f8 = dt.float8e4
    Exp = mybir.ActivationFunctionType.Exp
    DR = mybir.MatmulPerfMode.DoubleRow

    nc = bacc.Bacc("TRN2", target_bir_lowering=False, debug=False, num_devices=8)

    xb = nc.dram_tensor("xb", [C, N], f8, kind="ExternalInput")
    wq3 = nc.dram_tensor("wq3", [128, NCC, GS], f8, kind="ExternalInput")
    wk3 = nc.dram_tensor("wk3", [128, NCC, GS], f8, kind="ExternalInput")
    wv3 = nc.dram_tensor("wv3", [128, NCC, GS], f8, kind="ExternalInput")
    woT = nc.dram_tensor("woT", [GS, C], bf16, kind="ExternalInput")
    bq = nc.dram_tensor("bq", [GS, 1], f32, kind="ExternalInput")
    bk = nc.dram_tensor("bk", [GS, 1], f32, kind="ExternalInput")
    bvb = nc.dram_tensor("bvb", [128, GS], f32, kind="ExternalInput")
    outp = nc.dram_tensor("outp", [C, N], f32, kind="ExternalOutput")

    with tile.TileContext(nc) as tc, ExitStack() as ctx:
        persist = ctx.enter_context(tc.tile_pool(name="persist", bufs=1))
        epool = ctx.enter_context(tc.tile_pool(name="epool", bufs=4))
        # bufs=4: tail chains of consecutive q-tiles otherwise couple
        # through slot reuse and cascade-delay each other
        spool = ctx.enter_context(tc.tile_pool(name="spool", bufs=4))
        # PSUM budget (8 banks): psS 2x[128,2,512]=4, psO 2x[128,512]=2,
        # psD 1, psP 1.
        psS = ctx.enter_context(tc.tile_pool(name="psS", bufs=2, space="PSUM"))
        psO = ctx.enter_context(tc.tile_pool(name="psO", bufs=2, space="PSUM"))
        psD = ctx.enter_context(tc.tile_pool(name="psD", bufs=1, space="PSUM"))
        psP = ctx.enter_context(tc.tile_pool(name="psP", bufs=1, space="PSUM"))

        def load(shape, dtype, dram_ap, tag):
            t = persist.tile(shape, dtype, tag=tag)
            nc.sync.dma_start(t[:], dram_ap)
            return t

        # Weights first: they gate the first projection matmuls.
        wq_sb = load([128, NCC, GS], f8, wq3[:, :, :], "wq")
        wk_sb = load([128, NCC, GS], f8, wk3[:, :, :], "wk")
        wv_sb = load([128, NCC, GS], f8, wv3[:, :, :], "wv")
        wo_sb = load([GS, C], bf16, woT[:, :], "wo")
        bq_sb = load([GS, 1], f32, bq[:, :], "bq")
        bk_sb = load([GS, 1], f32, bk[:, :], "bk")
        bvb_sb = load([128, GS], f32, bvb[:, :], "bvb")

        xf = [None] * NQT

        def load_xf(nt):
            t = persist.tile([128, NCC, QT], f8, tag=f"xf{nt}")
            for cc in range(NCC):
                nc.sync.dma_start(
                    t[:, cc, :],
                    xb[cc * 128:(cc + 1) * 128, nt * QT:(nt + 1) * QT])
            xf[nt] = t

        # [128, 2, 16] with only column 0 used: the dual-fp8 ldweights
        # requires the outer free-AP step to be even and 16B-aligned,
        # so the two double-row planes must sit 16B apart.
        ones2 = persist.tile([128, 2, 16], f8, tag="ones2")
        nc.vector.memset(ones2[:], 1.0)
        ebias = persist.tile([128, 1], f32, tag="ebias")
        nc.vector.memset(ebias[:], EXP_BIAS)
        ones_1 = persist.tile([1, 128], bf16, tag="ones_1")
        nc.vector.memset(ones_1[:], 1.0)

        q_sb = persist.tile([GS, N], bf16, tag="q_sb")
        k_sb = persist.tile([GS, N], bf16, tag="k_sb")
        vt_sb = persist.tile([128, NKC, 128], f8, tag="vt_sb")

        # Projections, emitted per column block (nt) with its xf loads
        # inline so compute starts after ~4 DMAs and overlaps the rest.
        # Q/K: [gs, N] = W_g @ xf (+ bias per partition);
        # V^T: [keys, gs] per 128-key chunk = xf_chunk^T @ Wv_g^T.
        # All DoubleRow fp8: two 128-deep C-chunks per PE pass.
        load_xf(0)
        for nt in range(NQT):
            if nt + 1 < NQT:
                load_xf(nt + 1)
            nsl = slice(nt * QT, (nt + 1) * QT)
            for w_t, b_t, dst in ((wq_sb, bq_sb, q_sb), (wk_sb, bk_sb, k_sb)):
                ps = psO.tile([128, QT], f32, tag="po")
                for pr in range(NCC // 2):
                    nc.tensor.matmul(ps[:], w_t[:, 2 * pr:2 * pr + 2, :],
                                     xf[nt][:, 2 * pr:2 * pr + 2, :],
                                     start=(pr == 0), stop=(pr == NCC // 2 - 1),
                                     perf_mode=DR)
                nc.vector.tensor_scalar_add(dst[:, nsl], ps[:], b_t[:])
            for j in range(QT // 128):
                kc = nt * (QT // 128) + j
                off = j * 128
                ps = psS.tile([128, GS], f32, tag="ps")
                for pr in range(NCC // 2):
                    nc.tensor.matmul(ps[:], xf[nt][:, 2 * pr:2 * pr + 2,
                                                   off:off + 128],
                                     wv_sb[:, 2 * pr:2 * pr + 2, :],
                                     start=(pr == 0), stop=(pr == NCC // 2 - 1),
                                     perf_mode=DR)
                nc.vector.tensor_add(vt_sb[:, kc, :], ps[:], bvb_sb[:])

        # Attention, software-pipelined per query tile over PAIRS of exp
        # groups: PE emits S(p+1) while ScalarE runs exp(p), then the
        # accumulating O/D DoubleRow matmuls are batched same-bank
        # back-to-back ([O,O] then [D,D]).
        def emit_S(qt, g):
            qsl = slice(qt * QT, (qt + 1) * QT)
            ps = psS.tile([128, KG, QT], f32, tag="ps")
            for j in range(KG):
                kc = g * KG + j
                ksl = slice(kc * 128, (kc + 1) * 128)
                nc.tensor.matmul(ps[:, j, :],
                                 k_sb[:, ksl], q_sb[:, qsl],
                                 start=True, stop=True)
            return ps

        tails = []

        def emit_tail(qt, po, pd):
            state = {}

            def tail_pre():
                # free the pd bank + start the reciprocal chain early
                den_sb = spool.tile([1, QT], bf16, tag="den")
                nc.vector.tensor_copy(den_sb[:], pd[:])
                pb = psP.tile([128, QT], f32, tag="pp")
                nc.tensor.matmul(pb[:], ones_1[:], den_sb[:],
                                 start=True, stop=True)
                binv = spool.tile([128, QT], f32, tag="binv")
                nc.vector.reciprocal(binv[:], pb[:])
                state["binv"] = binv

            def tail_main():
                qsl = slice(qt * QT, (qt + 1) * QT)
                o_sb = spool.tile([128, QT], bf16, tag="osb")
                nc.vector.tensor_mul(o_sb[:], po[:], state["binv"][:])
                for mc in range(NMC):
                    msl = slice(mc * 128, (mc + 1) * 128)
                    pp = psP.tile([128, QT], f32, tag="pp")
                    nc.tensor.matmul(pp[:], wo_sb[:, msl], o_sb[:],
                                     start=True, stop=True)
                    st = spool.tile([128, QT], f32, tag="st")
                    nc.vector.tensor_copy(st[:], pp[:])
                    nc.sync.dma_start(outp[msl, qsl], st[:])
            return tail_pre, tail_main

        NP = NGR // 2            # 8 group pairs per query tile
        for qt in range(NQT):
            po = psO.tile([128, QT], f32, tag="po")
            s_cur = [emit_S(qt, 0), emit_S(qt, 1)]
            if tails:
                tails[-1][0]()         # prev epilogue: den copy + bcast + recip
            pd = psD.tile([1, QT], f32, tag="pd")
            for p in range(NP):
                # rest of prev epilogue one pair in: by now its DVE
                # chain is done, so the Wo matmuls don't stall PE
                if p == 1 and tails:
                    tails.pop()[1]()
                e_cur = []
                for i in range(2):
                    e = epool.tile([128, KG, QT], f8, tag="e")
                    nc.scalar.activation(e[:], s_cur[i][:], Exp,
                                         scale=SCALE / (WS * WS),
                                         bias=ebias[:])
                    e_cur.append(e)
                s_next = ([emit_S(qt, 2 * p + 2), emit_S(qt, 2 * p + 3)]
                          if p + 1 < NP else None)
                # same-psum-bank matmuls back-to-back: [O,O] then [D,D]
                for i in range(2):
                    g = 2 * p + i
                    nc.tensor.matmul(po[:], vt_sb[:, 2 * g:2 * g + 2, :],
                                     e_cur[i][:],
                                     start=(g == 0), stop=(g == NGR - 1),
                                     perf_mode=DR)
                for i in range(2):
                    g = 2 * p + i
                    nc.tensor.matmul(pd[:], ones2[:, :, 0:1], e_cur[i][:],
                                     start=(g == 0), stop=(g == NGR - 1),
                                     perf_mode=DR)
                s_cur = s_next
            tails.append(emit_tail(qt, po, pd))
        tp, tm = tails.pop()
        tp()
        tm()

    nc.compile()
    return nc


def _get_compiled():
    global _compiled_nc
    if _compiled_nc is None:
        _compiled_nc = _build()
    return _compiled_nc


def _ensure_ntff_hook():
    """Best-effort: register the axon NTFF profile hook so trace=True
    yields exec_time_ns. The image's antenv lacks axon_hooks; shim it."""
    import sys, types
    try:
        from antenv.axon_hooks import get_axon_ntff_profile_hook  # noqa: F401
        return
    except ImportError:
        pass
    try:
        mod = types.ModuleType("antenv.axon_hooks")
        _hook = [None]
        mod.set_axon_ntff_profile_hook = lambda h: _hook.__setitem__(0, h)
        mod.get_axon_ntff_profile_hook = lambda: _hook[0]
        sys.modules["antenv.axon_hooks"] = mod
        import antenv
        antenv.axon_hooks = mod
        from trn_agent_boot.trn_boot import _ntff_profile_via_ctypes
        mod.set_axon_ntff_profile_hook(
            _ntff_profile_via_ctypes("/opt/axon/libaxon_pjrt.so"))
    except Exception:
        pass


def kernel(x, Wq, bq, Wk, bk, Wv, bv, Wo, bo):
    global LAST_RESULT
    from concourse.bass_utils import run_bass_kernel_spmd

    nc = _get_compiled()
    f8 = ml_dtypes.float8_e4m3
    x = np.asarray(x, dtype=np.float32)
    b, c, d, h, w = x.shape
    n = d * h * w
    xf = x.reshape(b, c, n)
    Wq = np.asarray(Wq, np.float32)
    Wk = np.asarray(Wk, np.float32)
    Wv = np.asarray(Wv, np.float32)
    Wo = np.asarray(Wo, np.float32)
    bq = np.asarray(bq, np.float32)
    bk = np.asarray(bk, np.float32)
    bv = np.asarray(bv, np.float32)
    bo = np.asarray(bo, np.float32)

    def w3(Wm, gsl):
        # [128, NCC, GS] with w3[p, cc, m] = WS * Wm[g*GS+m, 128cc+p]
        a = (WS * Wm[gsl, :]).T            # [C, GS]
        return np.ascontiguousarray(
            a.reshape(NCC, 128, GS).transpose(1, 0, 2)).astype(f8)

    in_maps = []
    for core in range(8):
        bb, g = divmod(core, G)
        gsl = slice(g * GS, (g + 1) * GS)
        in_maps.append({
            "xb": np.ascontiguousarray(xf[bb]).astype(f8),
            "wq3": w3(Wq, gsl),
            "wk3": w3(Wk, gsl),
            "wv3": w3(Wv, gsl),
            "woT": np.ascontiguousarray(Wo[:, gsl].T / WS).astype(
                ml_dtypes.bfloat16),
            "bq": (WS * bq[gsl]).reshape(GS, 1).copy(),
            "bk": (WS * bk[gsl]).reshape(GS, 1).copy(),
            "bvb": np.ascontiguousarray(
                np.broadcast_to(WS * bv[gsl], (128, GS))).astype(np.float32),
        })

    trace = bool(os.environ.get("BASS_TRACE"))
    if trace:
        _ensure_ntff_hook()
    LAST_RESULT = run_bass_kernel_spmd(
        nc, in_maps, core_ids=list(range(8)), trace=trace)
    outs = LAST_RESULT.results

    out = np.empty((b, c, n), np.float32)
    for bb in range(b):
        acc = xf[bb] + bo[:, None]
        for g in range(G):
            acc = acc + outs[bb * G + g]["outp"]
        out[bb] = acc
    return out.reshape(b, c, d, h, w)


# revision 12
# speedup vs baseline: 1.4907x; 1.1376x over previous
"""Grouped-query attention kernel for 8 Trainium2 NeuronCores.

Problem (hardcoded): x [2, 512, 16, 16, 16] f32, Wq/Wk/Wv/Wo [512, 512],
biases [512]. G=4 heads of dim 128, N=4096 tokens. out = x + Wo@attn.

Sharding: one (batch, group) pair per core -> 8 cores, no cross-core
communication. Each core computes its group's Q/K/V projections, the
full 4096x4096 attention for its (b, g), and a partial output
projection Wo[:, g_cols] @ O_g -> [512, 4096]. Host sums the 4 partials
per batch and adds the residual + bo.

fp8 strategy: weights are pre-scaled x16 so their N(0, 1/sqrt(C))
entries land in fp8e4's normal range; x, Wq/Wk/Wv, V^T and the exp'd
scores all live in fp8e4 so every big matmul runs in DoubleRow perf
mode (two 128-deep contraction chunks per PE pass). The x16 scales are
unwound via the exp scale (/256) and Wo (/16 on host). exp also folds
a -2 bias (uniform across keys, cancels in softmax) to keep e^s < 240.

Device-side layout (per core):
  - xf (x[b] as [128, 4, 4096]) fp8, per q-tile chunks in SBUF
  - Q, K: [128(gs), 4096] bf16; V^T: [128, 32, 128] fp8
  - per query tile (512 wide), per pair of 256-key exp groups:
    S^T = K_chunk^T Q_tile -> PSUM (bf16 matmuls), exp on ScalarE
    (scale+bias folded) -> E fp8 [128, 2, 512], then DoubleRow
    accumulate O += V^T E and denom += ones^T E on TensorE, batching
    same-bank accumulating matmuls back-to-back ([O,O] then [D,D]).
    Normalize via reciprocal + ones-broadcast matmul, then Wo partial.
"""

import os
import numpy as np
import ml_dtypes

B, C, N, G = 2, 512, 4096, 4
GS = C // G          # 128 head dim
SCALE = GS ** -0.5
QT = 512             # query tile width
NQT = N // QT        # 8 query tiles
NKC = N // 128       # 32 key chunks
NCC = C // 128       # 4 contraction chunks for projections
NMC = C // 128       # 4 output-channel chunks
KG = 2               # key chunks per exp group (= DoubleRow pair)
NGR = NKC // KG      # 16 groups per query tile
WS = 16.0            # fp8 weight pre-scale
EXP_BIAS = -1.25     # uniform logit shift, cancels in softmax
# Schraudolph exp-as-fp8-bits on DVE: uint8 = round(A8*x + B8) with
# saturation, bitcast to fp8e4 approximates exp(x + EXP_BIAS) within
# ~3.4% rms (adj -0.45 tuned numerically; error cancels in softmax).
# Logits are in [-6.6, 6.5] so both exp paths stay under fp8e4's 240.
A8 = 8.0 / np.log(2.0)
B8 = 56.0 + A8 * EXP_BIAS - 0.45

_compiled_nc = None
LAST_RESULT = None


def _build():
    from contextlib import ExitStack
    import concourse.mybir as mybir
    import concourse.tile as tile
    from concourse import bacc

    dt = mybir.dt
    f32 = dt.float32
    bf16 = dt.bfloat16
    f8 = dt.float8e4
    u8 = dt.uint8
    Exp = mybir.ActivationFunctionType.Exp
    DR = mybir.MatmulPerfMode.DoubleRow
    Alu = mybir.AluOpType

    nc = bacc.Bacc("TRN2", target_bir_lowering=False, debug=False, num_devices=8)

    xb = nc.dram_tensor("xb", [C, N], f8, kind="ExternalInput")
    wq3 = nc.dram_tensor("wq3", [128, NCC, GS], f8, kind="ExternalInput")
    wk3 = nc.dram_tensor("wk3", [128, NCC, GS], f8, kind="ExternalInput")
    wv3 = nc.dram_tensor("wv3", [128, NCC, GS], f8, kind="ExternalInput")
    woT = nc.dram_tensor("woT", [GS, C], bf16, kind="ExternalInput")
    bq = nc.dram_tensor("bq", [GS, 1], f32, kind="ExternalInput")
    bk = nc.dram_tensor("bk", [GS, 1], f32, kind="ExternalInput")
    bvb = nc.dram_tensor("bvb", [128, GS], f32, kind="ExternalInput")
    outp = nc.dram_tensor("outp", [C, N], bf16, kind="ExternalOutput")

    with tile.TileContext(nc) as tc, ExitStack() as ctx:
        persist = ctx.enter_context(tc.tile_pool(name="persist", bufs=1))
        epool = ctx.enter_context(tc.tile_pool(name="epool", bufs=4))
        # bufs=4: tail chains of consecutive q-tiles otherwise couple
        # through slot reuse and cascade-delay each other
        spool = ctx.enter_context(tc.tile_pool(name="spool", bufs=4))
        # PSUM budget (8 banks): psS 2x[128,2,512]=4, psO 2x[128,512]=2,
        # psD 1, psP 1.
        psS = ctx.enter_context(tc.tile_pool(name="psS", bufs=2, space="PSUM"))
        psO = ctx.enter_context(tc.tile_pool(name="psO", bufs=2, space="PSUM"))
        psD = ctx.enter_context(tc.tile_pool(name="psD", bufs=1, space="PSUM"))
        psP = ctx.enter_context(tc.tile_pool(name="psP", bufs=1, space="PSUM"))

        def load(shape, dtype, dram_ap, tag):
            t = persist.tile(shape, dtype, tag=tag)
            nc.sync.dma_start(t[:], dram_ap)
            return t

        # Weights first: they gate the first projection matmuls.
        wq_sb = load([128, NCC, GS], f8, wq3[:, :, :], "wq")
        wk_sb = load([128, NCC, GS], f8, wk3[:, :, :], "wk")
        wv_sb = load([128, NCC, GS], f8, wv3[:, :, :], "wv")
        wo_sb = load([GS, C], bf16, woT[:, :], "wo")
        bq_sb = load([GS, 1], f32, bq[:, :], "bq")
        bk_sb = load([GS, 1], f32, bk[:, :], "bk")
        bvb_sb = load([128, GS], f32, bvb[:, :], "bvb")

        xf = [None] * NQT

        def load_xf(nt):
            t = persist.tile([128, NCC, QT], f8, tag=f"xf{nt}")
            for cc in range(NCC):
                nc.sync.dma_start(
                    t[:, cc, :],
                    xb[cc * 128:(cc + 1) * 128, nt * QT:(nt + 1) * QT])
            xf[nt] = t

        # [128, 2, 16] with only column 0 used: the dual-fp8 ldweights
        # requires the outer free-AP step to be even and 16B-aligned,
        # so the two double-row planes must sit 16B apart.
        ones2 = persist.tile([128, 2, 16], f8, tag="ones2")
        nc.vector.memset(ones2[:], 1.0)
        ebias = persist.tile([128, 1], f32, tag="ebias")
        nc.vector.memset(ebias[:], EXP_BIAS)
        ones_1 = persist.tile([1, 128], bf16, tag="ones_1")
        nc.vector.memset(ones_1[:], 1.0)

        q_sb = persist.tile([GS, N], bf16, tag="q_sb")
        k_sb = persist.tile([GS, N], bf16, tag="k_sb")
        vt_sb = persist.tile([128, NKC, 128], f8, tag="vt_sb")

        # Projections, emitted per column block (nt) with its xf loads
        # inline so compute starts after ~4 DMAs and overlaps the rest.
        # Q/K: [gs, N] = W_g @ xf (+ bias per partition);
        # V^T: [keys, gs] per 128-key chunk = xf_chunk^T @ Wv_g^T.
        # All DoubleRow fp8: two 128-deep C-chunks per PE pass.
        load_xf(0)
        for nt in range(NQT):
            if nt + 1 < NQT:
                load_xf(nt + 1)
            nsl = slice(nt * QT, (nt + 1) * QT)
            for w_t, b_t, dst in ((wq_sb, bq_sb, q_sb), (wk_sb, bk_sb, k_sb)):
                ps = psO.tile([128, QT], f32, tag="po")
                for pr in range(NCC // 2):
                    nc.tensor.matmul(ps[:], w_t[:, 2 * pr:2 * pr + 2, :],
                                     xf[nt][:, 2 * pr:2 * pr + 2, :],
                                     start=(pr == 0), stop=(pr == NCC // 2 - 1),
                                     perf_mode=DR)
                nc.vector.tensor_scalar_add(dst[:, nsl], ps[:], b_t[:])
            for j in range(QT // 128):
                kc = nt * (QT // 128) + j
                off = j * 128
                ps = psS.tile([128, GS], f32, tag="ps")
                for pr in range(NCC // 2):
                    nc.tensor.matmul(ps[:], xf[nt][:, 2 * pr:2 * pr + 2,
                                                   off:off + 128],
                                     wv_sb[:, 2 * pr:2 * pr + 2, :],
                                     start=(pr == 0), stop=(pr == NCC // 2 - 1),
                                     perf_mode=DR)
                nc.vector.tensor_add(vt_sb[:, kc, :], ps[:], bvb_sb[:])

        # Attention, software-pipelined per query tile over PAIRS of exp
        # groups: PE emits S(p+1) while ScalarE runs exp(p), then the
        # accumulating O/D DoubleRow matmuls are batched same-bank
        # back-to-back ([O,O] then [D,D]).
        def emit_S(qt, g):
            qsl = slice(qt * QT, (qt + 1) * QT)
            ps = psS.tile([128, KG, QT], f32, tag="ps")
            for j in range(KG):
                kc = g * KG + j
                ksl = slice(kc * 128, (kc + 1) * 128)
                nc.tensor.matmul(ps[:, j, :],
                                 k_sb[:, ksl], q_sb[:, qsl],
                                 start=True, stop=True)
            return ps

        tails = []

        def emit_tail(qt, po, pd):
            state = {}

            def tail_pre():
                # free the pd bank + start the reciprocal chain early
                den_sb = spool.tile([1, QT], bf16, tag="den")
                nc.scalar.copy(den_sb[:], pd[:])
                pb = psP.tile([128, QT], f32, tag="pp")
                nc.tensor.matmul(pb[:], ones_1[:], den_sb[:],
                                 start=True, stop=True)
                binv = spool.tile([128, QT], f32, tag="binv")
                nc.vector.reciprocal_approx_fast(binv[:], pb[:])
                state["binv"] = binv

            def tail_main():
                qsl = slice(qt * QT, (qt + 1) * QT)
                o_sb = spool.tile([128, QT], bf16, tag="osb")
                nc.vector.tensor_mul(o_sb[:], po[:], state["binv"][:])
                for mc in range(NMC):
                    msl = slice(mc * 128, (mc + 1) * 128)
                    pp = psP.tile([128, QT], f32, tag="pp")
                    nc.tensor.matmul(pp[:], wo_sb[:, msl], o_sb[:],
                                     start=True, stop=True)
                    st = spool.tile([128, QT], bf16, tag="st")
                    nc.scalar.copy(st[:], pp[:])
                    nc.sync.dma_start(outp[msl, qsl], st[:])
            return tail_pre, tail_main

        NP = NGR // 2            # 8 group pairs per query tile
        for qt in range(NQT):
            po = psO.tile([128, QT], f32, tag="po")
            s_cur = [emit_S(qt, 0), emit_S(qt, 1)]
            if tails:
                tails[-1][0]()         # prev epilogue: den copy + bcast + recip
            pd = psD.tile([1, QT], f32, tag="pd")
            for p in range(NP):
                # rest of prev epilogue one pair in: by now its DVE
                # chain is done, so the Wo matmuls don't stall PE
                if p == 1 and tails:
                    tails.pop()[1]()
                # exp split: group g0 exact on ScalarE (fp8 values),
                # g1 Schraudolph-bits on DVE (uint8, bitcast to fp8).
                e0 = epool.tile([128, KG, QT], f8, tag="e")
                nc.scalar.activation(e0[:], s_cur[0][:], Exp,
                                     scale=SCALE / (WS * WS),
                                     bias=ebias[:])
                e1 = epool.tile([128, KG, QT], u8, tag="e")
                nc.vector.tensor_scalar(e1[:], s_cur[1][:],
                                        A8 * SCALE / (WS * WS), B8,
                                        Alu.mult, Alu.add)
                e_cur = [e0[:], e1[:].bitcast(f8)]
                s_next = ([emit_S(qt, 2 * p + 2), emit_S(qt, 2 * p + 3)]
                          if p + 1 < NP else None)
                # same-psum-bank matmuls back-to-back: [O,O] then [D,D]
                for i in range(2):
                    g = 2 * p + i
                    nc.tensor.matmul(po[:], vt_sb[:, 2 * g:2 * g + 2, :],
                                     e_cur[i],
                                     start=(g == 0), stop=(g == NGR - 1),
                                     perf_mode=DR)
                for i in range(2):
                    g = 2 * p + i
                    nc.tensor.matmul(pd[:], ones2[:, :, 0:1], e_cur[i],
                                     start=(g == 0), stop=(g == NGR - 1),
                                     perf_mode=DR)
                s_cur = s_next
            tails.append(emit_tail(qt, po, pd))
        tp, tm = tails.pop()
        tp()
        tm()

    nc.compile()
    return nc


def _get_compiled():
    global _compiled_nc
    if _compiled_nc is None:
        _compiled_nc = _build()
    return _compiled_nc


def _ensure_ntff_hook():
    """Best-effort: register the axon NTFF profile hook so trace=True
    yields exec_time_ns. The image's antenv lacks axon_hooks; shim it."""
    import sys, types
    try:
        from antenv.axon_hooks import get_axon_ntff_profile_hook  # noqa: F401
        return
    except ImportError:
        pass
    try:
        mod = types.ModuleType("antenv.axon_hooks")
        _hook = [None]
        mod.set_axon_ntff_profile_hook = lambda h: _hook.__setitem__(0, h)
        mod.get_axon_ntff_profile_hook = lambda: _hook[0]
        sys.modules["antenv.axon_hooks"] = mod
        import antenv
        antenv.axon_hooks = mod
        from trn_agent_boot.trn_boot import _ntff_profile_via_ctypes
        mod.set_axon_ntff_profile_hook(
            _ntff_profile_via_ctypes("/opt/axon/libaxon_pjrt.so"))
    except Exception:
        pass


def kernel(x, Wq, bq, Wk, bk, Wv, bv, Wo, bo):
    global LAST_RESULT
    from concourse.bass_utils import run_bass_kernel_spmd

    nc = _get_compiled()
    f8 = ml_dtypes.float8_e4m3
    x = np.asarray(x, dtype=np.float32)
    b, c, d, h, w = x.shape
    n = d * h * w
    xf = x.reshape(b, c, n)
    Wq = np.asarray(Wq, np.float32)
    Wk = np.asarray(Wk, np.float32)
    Wv = np.asarray(Wv, np.float32)
    Wo = np.asarray(Wo, np.float32)
    bq = np.asarray(bq, np.float32)
    bk = np.asarray(bk, np.float32)
    bv = np.asarray(bv, np.float32)
    bo = np.asarray(bo, np.float32)

    def w3(Wm, gsl):
        # [128, NCC, GS] with w3[p, cc, m] = WS * Wm[g*GS+m, 128cc+p]
        a = (WS * Wm[gsl, :]).T            # [C, GS]
        return np.ascontiguousarray(
            a.reshape(NCC, 128, GS).transpose(1, 0, 2)).astype(f8)

    in_maps = []
    for core in range(8):
        bb, g = divmod(core, G)
        gsl = slice(g * GS, (g + 1) * GS)
        in_maps.append({
            "xb": np.ascontiguousarray(xf[bb]).astype(f8),
            "wq3": w3(Wq, gsl),
            "wk3": w3(Wk, gsl),
            "wv3": w3(Wv, gsl),
            "woT": np.ascontiguousarray(Wo[:, gsl].T / WS).astype(
                ml_dtypes.bfloat16),
            "bq": (WS * bq[gsl]).reshape(GS, 1).copy(),
            "bk": (WS * bk[gsl]).reshape(GS, 1).copy(),
            "bvb": np.ascontiguousarray(
                np.broadcast_to(WS * bv[gsl], (128, GS))).astype(np.float32),
        })

    trace = bool(os.environ.get("BASS_TRACE"))
    if trace:
        _ensure_ntff_hook()
    LAST_RESULT = run_bass_kernel_spmd(
        nc, in_maps, core_ids=list(range(8)), trace=trace)
    outs = LAST_RESULT.results

    out = np.empty((b, c, n), np.float32)
    for bb in range(b):
        acc = xf[bb] + bo[:, None]
        for g in range(G):
            acc = acc + outs[bb * G + g]["outp"].astype(np.float32)
        out[bb] = acc
    return out.reshape(b, c, d, h, w)


# revision 16
# speedup vs baseline: 1.6561x; 1.1110x over previous
"""Grouped-query attention kernel for 8 Trainium2 NeuronCores.

Problem (hardcoded): x [2, 512, 16, 16, 16] f32, Wq/Wk/Wv/Wo [512, 512],
biases [512]. G=4 heads of dim 128, N=4096 tokens. out = x + Wo@attn.

Sharding: one (batch, group) pair per core -> 8 cores, no cross-core
communication. Each core computes its group's Q/K/V projections, the
full 4096x4096 attention for its (b, g), an UNNORMALIZED output
projection Wo[:, g_cols] @ (E V) -> [512, 4096] plus the softmax
denominator row [4096]. The host divides each partial by its core's
denominator, sums the 4 partials per batch and adds residual + bo
(softmax normalization commutes with the linear Wo).

fp8 strategy: weights are pre-scaled x16 so their N(0, 1/sqrt(C))
entries land in fp8e4's normal range; x, Wq/Wk/Wv, V^T and the exp'd
scores all live in fp8e4 so the projection and PV/denominator matmuls
run in DoubleRow perf mode (two 128-deep contraction chunks per PE
pass = 2x). The x16 scales are unwound via the exp scale (/256) and
Wo (/16 on host). exp folds a -1.25 bias (uniform across keys, cancels
in softmax) to keep e^s inside fp8e4 range.

exp is split across engines per pair of 256-key groups: group 0 exact
on ScalarE (fp8 out), group 1 on DVE as Schraudolph bits (affine to
uint8 with round+saturate, bitcast to fp8e4). S results live in four
1-bank PSUM chunk tiles with one exp instruction per chunk, so the
slot round-trip (S matmul -> exp -> reuse) is shorter than the PE
pair period and the PE never waits on the exp engines.
"""

import os
import numpy as np
import ml_dtypes

B, C, N, G = 2, 512, 4096, 4
GS = C // G          # 128 head dim
SCALE = GS ** -0.5
QT = 512             # query tile width
NQT = N // QT        # 8 query tiles
NKC = N // 128       # 32 key chunks
NCC = C // 128       # 4 contraction chunks for projections
NMC = C // 128       # 4 output-channel chunks
KG = 2               # key chunks per exp group (= DoubleRow pair)
NGR = NKC // KG      # 16 groups per query tile
NP = NGR // 2        # 8 group pairs per query tile
WS = 16.0            # fp8 weight pre-scale
EXP_BIAS = -1.25     # uniform logit shift, cancels in softmax
# Schraudolph exp-as-fp8-bits on DVE: uint8 = round(A8*x + B8) with
# saturation, bitcast to fp8e4 approximates exp(x + EXP_BIAS) within
# ~3.4% rms (adj -0.45 tuned numerically; error cancels in softmax).
# Logits are in [-6.6, 6.5] so both exp paths stay under fp8e4's 240.
A8 = 8.0 / np.log(2.0)
B8 = 56.0 + A8 * EXP_BIAS - 0.45

_compiled_nc = None
LAST_RESULT = None


def _build():
    from contextlib import ExitStack
    import concourse.mybir as mybir
    import concourse.tile as tile
    from concourse import bacc

    dt = mybir.dt
    f32 = dt.float32
    bf16 = dt.bfloat16
    f8 = dt.float8e4
    u8 = dt.uint8
    Exp = mybir.ActivationFunctionType.Exp
    DR = mybir.MatmulPerfMode.DoubleRow
    Alu = mybir.AluOpType

    nc = bacc.Bacc("TRN2", target_bir_lowering=False, debug=False, num_devices=8)

    xb = nc.dram_tensor("xb", [C, N], f8, kind="ExternalInput")
    wq3 = nc.dram_tensor("wq3", [128, NCC, GS], f8, kind="ExternalInput")
    wk3 = nc.dram_tensor("wk3", [128, NCC, GS], f8, kind="ExternalInput")
    wv3 = nc.dram_tensor("wv3", [128, NCC, GS], f8, kind="ExternalInput")
    woT = nc.dram_tensor("woT", [GS, C], bf16, kind="ExternalInput")
    bq = nc.dram_tensor("bq", [GS, 1], f32, kind="ExternalInput")
    bk = nc.dram_tensor("bk", [GS, 1], f32, kind="ExternalInput")
    bvb = nc.dram_tensor("bvb", [128, GS], f32, kind="ExternalInput")
    outp = nc.dram_tensor("outp", [C, N], bf16, kind="ExternalOutput")
    denp = nc.dram_tensor("denp", [1, N], f32, kind="ExternalOutput")

    with tile.TileContext(nc) as tc, ExitStack() as ctx:
        persist = ctx.enter_context(tc.tile_pool(name="persist", bufs=1))
        epool = ctx.enter_context(tc.tile_pool(name="epool", bufs=4))
        spool = ctx.enter_context(tc.tile_pool(name="spool", bufs=4))
        # PSUM budget (8 banks): psS 4x[128,512]=4, psO 2x[128,512]=2,
        # psD 1 (pd, only partition 0), psP 1 (Wo outputs).
        psS = ctx.enter_context(tc.tile_pool(name="psS", bufs=4, space="PSUM"))
        psO = ctx.enter_context(tc.tile_pool(name="psO", bufs=2, space="PSUM"))
        psD = ctx.enter_context(tc.tile_pool(name="psD", bufs=1, space="PSUM"))
        psP = ctx.enter_context(tc.tile_pool(name="psP", bufs=1, space="PSUM"))

        def load(shape, dtype, dram_ap, tag):
            t = persist.tile(shape, dtype, tag=tag)
            nc.sync.dma_start(t[:], dram_ap)
            return t

        # Weights first: they gate the first projection matmuls.
        wq_sb = load([128, NCC, GS], f8, wq3[:, :, :], "wq")
        wk_sb = load([128, NCC, GS], f8, wk3[:, :, :], "wk")
        wv_sb = load([128, NCC, GS], f8, wv3[:, :, :], "wv")
        wo_sb = load([GS, C], bf16, woT[:, :], "wo")
        bq_sb = load([GS, 1], f32, bq[:, :], "bq")
        bk_sb = load([GS, 1], f32, bk[:, :], "bk")
        bvb_sb = load([128, GS], f32, bvb[:, :], "bvb")

        xf = [None] * NQT

        def load_xf(nt):
            t = persist.tile([128, NCC, QT], f8, tag=f"xf{nt}")
            for cc in range(NCC):
                nc.sync.dma_start(
                    t[:, cc, :],
                    xb[cc * 128:(cc + 1) * 128, nt * QT:(nt + 1) * QT])
            xf[nt] = t

        # [128, 2, 16] with only column 0 used: the dual-fp8 ldweights
        # requires the outer free-AP step to be even and 16B-aligned,
        # so the two double-row planes must sit 16B apart.
        ones2 = persist.tile([128, 2, 16], f8, tag="ones2")
        nc.vector.memset(ones2[:], 1.0)
        ebias = persist.tile([128, 1], f32, tag="ebias")
        nc.vector.memset(ebias[:], EXP_BIAS)

        q_sb = persist.tile([GS, N], bf16, tag="q_sb")
        k_sb = persist.tile([GS, N], bf16, tag="k_sb")
        vt_sb = persist.tile([128, NKC, 128], f8, tag="vt_sb")

        # Projections, emitted per column block (nt) with its xf loads
        # inline so compute starts after ~4 DMAs and overlaps the rest.
        # Q/K: [gs, N] = W_g @ xf (+ bias per partition, on ScalarE);
        # V^T: [keys, gs] per 128-key chunk = xf_chunk^T @ Wv_g^T (+
        # bias on DVE). All matmuls DoubleRow fp8.
        load_xf(0)
        for nt in range(NQT):
            if nt + 1 < NQT:
                load_xf(nt + 1)
            nsl = slice(nt * QT, (nt + 1) * QT)
            for w_t, b_t, dst in ((wq_sb, bq_sb, q_sb), (wk_sb, bk_sb, k_sb)):
                ps = psO.tile([128, QT], f32, tag="po")
                for pr in range(NCC // 2):
                    nc.tensor.matmul(ps[:], w_t[:, 2 * pr:2 * pr + 2, :],
                                     xf[nt][:, 2 * pr:2 * pr + 2, :],
                                     start=(pr == 0), stop=(pr == NCC // 2 - 1),
                                     perf_mode=DR)
                nc.scalar.add(dst[:, nsl], ps[:], b_t[:])
            for j in range(QT // 128):
                kc = nt * (QT // 128) + j
                off = j * 128
                ps = psS.tile([128, GS], f32, tag="ps")
                for pr in range(NCC // 2):
                    nc.tensor.matmul(ps[:], xf[nt][:, 2 * pr:2 * pr + 2,
                                                   off:off + 128],
                                     wv_sb[:, 2 * pr:2 * pr + 2, :],
                                     start=(pr == 0), stop=(pr == NCC // 2 - 1),
                                     perf_mode=DR)
                nc.vector.tensor_add(vt_sb[:, kc, :], ps[:], bvb_sb[:])

        # Attention, software-pipelined per query tile over PAIRS of
        # 256-key exp groups. Each group's S^T lands in two 1-bank PSUM
        # chunk tiles consumed by per-chunk exp instructions (ScalarE
        # for group 0, DVE Schraudolph for group 1), then the
        # accumulating O/D DoubleRow matmuls run same-bank back-to-back.
        def emit_S(qt, g):
            qsl = slice(qt * QT, (qt + 1) * QT)
            out = []
            for j in range(KG):
                kc = g * KG + j
                ksl = slice(kc * 128, (kc + 1) * 128)
                ps = psS.tile([128, QT], f32, tag="ps")
                nc.tensor.matmul(ps[:], k_sb[:, ksl], q_sb[:, qsl],
                                 start=True, stop=True)
                out.append(ps)
            return out

        tails = []

        def emit_tail(qt, po, pd):
            qsl = slice(qt * QT, (qt + 1) * QT)
            state = {}

            def pre():
                # free the pd + po banks; normalization happens on host
                den_sb = spool.tile([1, QT], f32, tag="den")
                nc.scalar.copy(den_sb[:], pd[:])
                nc.sync.dma_start(denp[0:1, qsl], den_sb[:])
                o_sb = spool.tile([128, QT], bf16, tag="osb")
                nc.scalar.copy(o_sb[:], po[:])
                state["o"] = o_sb

            def mk(mc):
                def f():
                    msl = slice(mc * 128, (mc + 1) * 128)
                    pp = psP.tile([128, QT], f32, tag="pp")
                    nc.tensor.matmul(pp[:], wo_sb[:, msl], state["o"][:],
                                     start=True, stop=True)
                    st = spool.tile([128, QT], bf16, tag="st")
                    if mc % 2 == 0:
                        nc.scalar.copy(st[:], pp[:])
                    else:
                        nc.vector.tensor_copy(st[:], pp[:])
                    nc.sync.dma_start(outp[msl, qsl], st[:])
                return f
            return pre, [mk(mc) for mc in range(NMC)]

        for qt in range(NQT):
            po = psO.tile([128, QT], f32, tag="po")
            s_cur = [emit_S(qt, 0), emit_S(qt, 1)]
            if tails:
                tails[-1][0]()     # prev tile: den + o copies (frees banks)
            pd = psD.tile([1, QT], f32, tag="pd")
            for p in range(NP):
                # prev tile's Wo matmuls, one per pair so the psP
                # bank round-trip (mm -> copy -> DMA) never stalls PE
                if tails and 1 <= p <= NMC:
                    tails[-1][1][p - 1]()
                    if p == NMC:
                        tails.pop()
                # exp split: group g0 exact on ScalarE (fp8 values),
                # g1 Schraudolph-bits on DVE; one instruction per chunk
                e0 = epool.tile([128, KG, QT], f8, tag="e")
                for j in range(KG):
                    nc.scalar.activation(e0[:, j, :], s_cur[0][j][:], Exp,
                                         scale=SCALE / (WS * WS),
                                         bias=ebias[:])
                e1 = epool.tile([128, KG, QT], u8, tag="e")
                for j in range(KG):
                    nc.vector.tensor_scalar(e1[:, j, :], s_cur[1][j][:],
                                            A8 * SCALE / (WS * WS), B8,
                                            Alu.mult, Alu.add)
                e_cur = [e0[:], e1[:].bitcast(f8)]
                s_next = ([emit_S(qt, 2 * p + 2), emit_S(qt, 2 * p + 3)]
                          if p + 1 < NP else None)
                # same-psum-bank matmuls back-to-back: [O,O] then [D,D]
                for i in range(2):
                    g = 2 * p + i
                    nc.tensor.matmul(po[:], vt_sb[:, 2 * g:2 * g + 2, :],
                                     e_cur[i],
                                     start=(g == 0), stop=(g == NGR - 1),
                                     perf_mode=DR)
                for i in range(2):
                    g = 2 * p + i
                    nc.tensor.matmul(pd[:], ones2[:, :, 0:1], e_cur[i],
                                     start=(g == 0), stop=(g == NGR - 1),
                                     perf_mode=DR)
                s_cur = s_next
            tails.append(emit_tail(qt, po, pd))
        tp, tms = tails.pop()
        tp()
        for f in tms:
            f()

    nc.compile()
    return nc


def _get_compiled():
    global _compiled_nc
    if _compiled_nc is None:
        _compiled_nc = _build()
    return _compiled_nc


def _ensure_ntff_hook():
    """Best-effort: register the axon NTFF profile hook so trace=True
    yields exec_time_ns. The image's antenv lacks axon_hooks; shim it."""
    import sys, types
    try:
        from antenv.axon_hooks import get_axon_ntff_profile_hook  # noqa: F401
        return
    except ImportError:
        pass
    try:
        mod = types.ModuleType("antenv.axon_hooks")
        _hook = [None]
        mod.set_axon_ntff_profile_hook = lambda h: _hook.__setitem__(0, h)
        mod.get_axon_ntff_profile_hook = lambda: _hook[0]
        sys.modules["antenv.axon_hooks"] = mod
        import antenv
        antenv.axon_hooks = mod
        from trn_agent_boot.trn_boot import _ntff_profile_via_ctypes
        mod.set_axon_ntff_profile_hook(
            _ntff_profile_via_ctypes("/opt/axon/libaxon_pjrt.so"))
    except Exception:
        pass


def kernel(x, Wq, bq, Wk, bk, Wv, bv, Wo, bo):
    global LAST_RESULT
    from concourse.bass_utils import run_bass_kernel_spmd

    nc = _get_compiled()
    f8 = ml_dtypes.float8_e4m3
    x = np.asarray(x, dtype=np.float32)
    b, c, d, h, w = x.shape
    n = d * h * w
    xf = x.reshape(b, c, n)
    Wq = np.asarray(Wq, np.float32)
    Wk = np.asarray(Wk, np.float32)
    Wv = np.asarray(Wv, np.float32)
    Wo = np.asarray(Wo, np.float32)
    bq = np.asarray(bq, np.float32)
    bk = np.asarray(bk, np.float32)
    bv = np.asarray(bv, np.float32)
    bo = np.asarray(bo, np.float32)

    def w3(Wm, gsl):
        # [128, NCC, GS] with w3[p, cc, m] = WS * Wm[g*GS+m, 128cc+p]
        a = (WS * Wm[gsl, :]).T            # [C, GS]
        return np.ascontiguousarray(
            a.reshape(NCC, 128, GS).transpose(1, 0, 2)).astype(f8)

    in_maps = []
    for core in range(8):
        bb, g = divmod(core, G)
        gsl = slice(g * GS, (g + 1) * GS)
        in_maps.append({
            "xb": np.ascontiguousarray(xf[bb]).astype(f8),
            "wq3": w3(Wq, gsl),
            "wk3": w3(Wk, gsl),
            "wv3": w3(Wv, gsl),
            "woT": np.ascontiguousarray(Wo[:, gsl].T / WS).astype(
                ml_dtypes.bfloat16),
            "bq": (WS * bq[gsl]).reshape(GS, 1).copy(),
            "bk": (WS * bk[gsl]).reshape(GS, 1).copy(),
            "bvb": np.ascontiguousarray(
                np.broadcast_to(WS * bv[gsl], (128, GS))).astype(np.float32),
        })

    trace = bool(os.environ.get("BASS_TRACE"))
    if trace:
        _ensure_ntff_hook()
    LAST_RESULT = run_bass_kernel_spmd(
        nc, in_maps, core_ids=list(range(8)), trace=trace)
    outs = LAST_RESULT.results

    out = np.empty((b, c, n), np.float32)
    for bb in range(b):
        acc = xf[bb] + bo[:, None]
        for g in range(G):
            o = outs[bb * G + g]
            den = o["denp"][0].astype(np.float32)
            acc = acc + o["outp"].astype(np.float32) / den[None, :]
        out[bb] = acc
    return out.reshape(b, c, d, h, w)


# revision 22
# speedup vs baseline: 1.7341x; 1.0471x over previous
"""Grouped-query attention kernel for 8 Trainium2 NeuronCores.

Problem (hardcoded): x [2, 512, 16, 16, 16] f32, Wq/Wk/Wv/Wo [512, 512],
biases [512]. G=4 heads of dim 128, N=4096 tokens. out = x + Wo@attn.

Sharding: one (batch, group) pair per core -> 8 cores, no cross-core
communication. Each core computes its group's Q/K/V projections, the
full 4096x4096 attention for its (b, g), an UNNORMALIZED output
projection Wo[:, g_cols] @ (E V) -> [512, 4096] plus the softmax
denominator row [4096]. The host divides each partial by its core's
denominator, sums the 4 partials per batch and adds residual + bo
(softmax normalization commutes with the linear Wo).

fp8 strategy: weights are pre-scaled x16 so their N(0, 1/sqrt(C))
entries land in fp8e4's normal range; x, Wq/Wk/Wv, V^T and the exp'd
scores all live in fp8e4 so the projection and PV/denominator matmuls
run in DoubleRow perf mode (two 128-deep contraction chunks per PE
pass = 2x). The x16 scales are unwound via the exp scale (/256) and
Wo (/16 on host). exp folds a -1.25 bias (uniform across keys, cancels
in softmax) to keep e^s inside fp8e4 range.

exp is split across engines per pair of 256-key groups: group 0 exact
on ScalarE (fp8 out), group 1 on DVE as Schraudolph bits (affine to
uint8 with round+saturate, bitcast to fp8e4). S results live in four
1-bank PSUM chunk tiles with one exp instruction per chunk, so the
slot round-trip (S matmul -> exp -> reuse) is shorter than the PE
pair period and the PE never waits on the exp engines.
"""

import os
import numpy as np
import ml_dtypes

B, C, N, G = 2, 512, 4096, 4
GS = C // G          # 128 head dim
SCALE = GS ** -0.5
QT = 512             # query tile width
NQT = N // QT        # 8 query tiles
NKC = N // 128       # 32 key chunks
NCC = C // 128       # 4 contraction chunks for projections
NMC = C // 128       # 4 output-channel chunks
KG = 2               # key chunks per exp group (= DoubleRow pair)
NGR = NKC // KG      # 16 groups per query tile
NP = NGR // 2        # 8 group pairs per query tile
WS = 16.0            # fp8 weight pre-scale
EXP_BIAS = -1.25     # uniform logit shift, cancels in softmax
# Schraudolph exp-as-fp8-bits on DVE: uint8 = round(A8*x + B8) with
# saturation, bitcast to fp8e4 approximates exp(x + EXP_BIAS) within
# ~3.4% rms (adj -0.45 tuned numerically; error cancels in softmax).
# Logits are in [-6.6, 6.5] so both exp paths stay under fp8e4's 240.
A8 = 8.0 / np.log(2.0)
B8 = 56.0 + A8 * EXP_BIAS - 0.45

_compiled_nc = None
LAST_RESULT = None


def _build():
    from contextlib import ExitStack
    import concourse.mybir as mybir
    import concourse.tile as tile
    from concourse import bacc

    dt = mybir.dt
    f32 = dt.float32
    bf16 = dt.bfloat16
    f8 = dt.float8e4
    u8 = dt.uint8
    Exp = mybir.ActivationFunctionType.Exp
    DR = mybir.MatmulPerfMode.DoubleRow
    Alu = mybir.AluOpType

    nc = bacc.Bacc("TRN2", target_bir_lowering=False, debug=False, num_devices=8)

    xb = nc.dram_tensor("xb", [C, N], f8, kind="ExternalInput")
    wq3 = nc.dram_tensor("wq3", [128, NCC, GS], f8, kind="ExternalInput")
    wk3 = nc.dram_tensor("wk3", [128, NCC, GS], f8, kind="ExternalInput")
    wv3 = nc.dram_tensor("wv3", [128, NCC, GS], f8, kind="ExternalInput")
    woT = nc.dram_tensor("woT", [GS, C], bf16, kind="ExternalInput")
    bq = nc.dram_tensor("bq", [GS, 1], f32, kind="ExternalInput")
    bk = nc.dram_tensor("bk", [GS, 1], f32, kind="ExternalInput")
    bvb = nc.dram_tensor("bvb", [128, GS], f32, kind="ExternalInput")
    outp = nc.dram_tensor("outp", [C, N], bf16, kind="ExternalOutput")
    denp = nc.dram_tensor("denp", [1, N], f32, kind="ExternalOutput")

    with tile.TileContext(nc) as tc, ExitStack() as ctx:
        persist = ctx.enter_context(tc.tile_pool(name="persist", bufs=1))
        epool = ctx.enter_context(tc.tile_pool(name="epool", bufs=4))
        spool = ctx.enter_context(tc.tile_pool(name="spool", bufs=4))
        # PSUM budget (8 banks): psS 4x[128,512]=4, psO 2x[128,512]=2,
        # psD 1 (pd, only partition 0), psP 1 (Wo outputs).
        psS = ctx.enter_context(tc.tile_pool(name="psS", bufs=4, space="PSUM"))
        psO = ctx.enter_context(tc.tile_pool(name="psO", bufs=2, space="PSUM"))
        psD = ctx.enter_context(tc.tile_pool(name="psD", bufs=1, space="PSUM"))
        psP = ctx.enter_context(tc.tile_pool(name="psP", bufs=1, space="PSUM"))

        # DMA triggers cost ~600ns of issue time on the issuing engine's
        # queue; round-robin them so loads don't serialize on one queue.
        dmaq = [nc.sync, nc.gpsimd, nc.sync, nc.gpsimd]
        wq_dmaq = [nc.sync, nc.gpsimd, nc.scalar]
        _dq = [0]

        def dma_rr(dst, src):
            wq_dmaq[_dq[0] % len(wq_dmaq)].dma_start(dst, src)
            _dq[0] += 1

        def load(shape, dtype, dram_ap, tag):
            t = persist.tile(shape, dtype, tag=tag)
            dma_rr(t[:], dram_ap)
            return t

        # Weights first: they gate the first projection matmuls.
        wq_sb = load([128, NCC, GS], f8, wq3[:, :, :], "wq")
        wk_sb = load([128, NCC, GS], f8, wk3[:, :, :], "wk")
        wv_sb = load([128, NCC, GS], f8, wv3[:, :, :], "wv")
        wo_sb = load([GS, C], bf16, woT[:, :], "wo")
        bq_sb = load([GS, 1], f32, bq[:, :], "bq")
        bk_sb = load([GS, 1], f32, bk[:, :], "bk")
        bvb_sb = load([128, GS], f32, bvb[:, :], "bvb")

        xf = [None] * NQT

        def load_xf(nt):
            t = persist.tile([128, NCC, QT], f8, tag=f"xf{nt}")
            for cc in range(NCC):
                dmaq[cc].dma_start(
                    t[:, cc, :],
                    xb[cc * 128:(cc + 1) * 128, nt * QT:(nt + 1) * QT])
            xf[nt] = t

        # [128, 2, 16] with only column 0 used: the dual-fp8 ldweights
        # requires the outer free-AP step to be even and 16B-aligned,
        # so the two double-row planes must sit 16B apart.
        ones2 = persist.tile([128, 2, 16], f8, tag="ones2")
        nc.vector.memset(ones2[:], 1.0)
        ebias = persist.tile([128, 1], f32, tag="ebias")
        nc.vector.memset(ebias[:], EXP_BIAS)

        q_sb = persist.tile([GS, N], bf16, tag="q_sb")
        k_sb = persist.tile([GS, N], bf16, tag="k_sb")
        vt_sb = persist.tile([128, NKC, 128], f8, tag="vt_sb")

        # Projections, emitted per column block (nt) with its xf loads
        # inline so compute starts after ~4 DMAs and overlaps the rest.
        # Q/K: [gs, N] = W_g @ xf (+ bias per partition, on ScalarE);
        # V^T: [keys, gs] per 128-key chunk = xf_chunk^T @ Wv_g^T (+
        # bias on DVE). All matmuls DoubleRow fp8.
        load_xf(0)
        for nt in range(NQT):
            if nt + 1 < NQT:
                load_xf(nt + 1)
            nsl = slice(nt * QT, (nt + 1) * QT)
            for w_t, b_t, dst in ((wq_sb, bq_sb, q_sb), (wk_sb, bk_sb, k_sb)):
                ps = psO.tile([128, QT], f32, tag="po")
                for pr in range(NCC // 2):
                    nc.tensor.matmul(ps[:], w_t[:, 2 * pr:2 * pr + 2, :],
                                     xf[nt][:, 2 * pr:2 * pr + 2, :],
                                     start=(pr == 0), stop=(pr == NCC // 2 - 1),
                                     perf_mode=DR)
                nc.scalar.add(dst[:, nsl], ps[:], b_t[:])
            for j in range(QT // 128):
                kc = nt * (QT // 128) + j
                off = j * 128
                ps = psS.tile([128, GS], f32, tag="ps")
                for pr in range(NCC // 2):
                    nc.tensor.matmul(ps[:], xf[nt][:, 2 * pr:2 * pr + 2,
                                                   off:off + 128],
                                     wv_sb[:, 2 * pr:2 * pr + 2, :],
                                     start=(pr == 0), stop=(pr == NCC // 2 - 1),
                                     perf_mode=DR)
                nc.vector.tensor_add(vt_sb[:, kc, :], ps[:], bvb_sb[:])

        # Attention, software-pipelined per query tile over PAIRS of
        # 256-key exp groups. Each group's S^T lands in two 1-bank PSUM
        # chunk tiles consumed by per-chunk exp instructions (ScalarE
        # for group 0, DVE Schraudolph for group 1), then the
        # accumulating O/D DoubleRow matmuls run same-bank back-to-back.
        def emit_S(qt, g):
            qsl = slice(qt * QT, (qt + 1) * QT)
            out = []
            for j in range(KG):
                kc = g * KG + j
                ksl = slice(kc * 128, (kc + 1) * 128)
                ps = psS.tile([128, QT], f32, tag="ps")
                nc.tensor.matmul(ps[:], k_sb[:, ksl], q_sb[:, qsl],
                                 start=True, stop=True)
                out.append(ps)
            return out

        tails = []

        def emit_tail(qt, po, pd):
            qsl = slice(qt * QT, (qt + 1) * QT)
            state = {}

            def pre():
                # free the pd + po banks; normalization happens on host
                den_sb = spool.tile([1, QT], f32, tag="den")
                nc.scalar.copy(den_sb[:], pd[:])
                nc.sync.dma_start(denp[0:1, qsl], den_sb[:])
                o_sb = spool.tile([128, QT], bf16, tag="osb")
                nc.scalar.copy(o_sb[:], po[:])
                state["o"] = o_sb

            def mk(mc):
                def f():
                    msl = slice(mc * 128, (mc + 1) * 128)
                    pp = psP.tile([128, QT], f32, tag="pp")
                    nc.tensor.matmul(pp[:], wo_sb[:, msl], state["o"][:],
                                     start=True, stop=True)
                    st = spool.tile([128, QT], bf16, tag="st")
                    if mc % 2 == 0:
                        nc.scalar.copy(st[:], pp[:])
                    else:
                        nc.vector.tensor_copy(st[:], pp[:])
                    nc.sync.dma_start(outp[msl, qsl], st[:])
                return f
            return pre, [mk(mc) for mc in range(NMC)]

        for qt in range(NQT):
            po = psO.tile([128, QT], f32, tag="po")
            s_cur = [emit_S(qt, 0), emit_S(qt, 1)]
            if tails:
                tails[-1][0]()     # prev tile: den + o copies (frees banks)
            pd = psD.tile([1, QT], f32, tag="pd")
            for p in range(NP):
                # prev tile's Wo matmuls, one per pair so the psP
                # bank round-trip (mm -> copy -> DMA) never stalls PE
                if tails and 1 <= p <= NMC:
                    tails[-1][1][p - 1]()
                    if p == NMC:
                        tails.pop()
                # exp split: group g0 exact on ScalarE (fp8 values),
                # g1 Schraudolph-bits on DVE; one instruction per chunk
                e0 = epool.tile([128, KG, QT], f8, tag="e")
                for j in range(KG):
                    nc.scalar.activation(e0[:, j, :], s_cur[0][j][:], Exp,
                                         scale=SCALE / (WS * WS),
                                         bias=ebias[:])
                e1 = epool.tile([128, KG, QT], u8, tag="e")
                for j in range(KG):
                    nc.vector.tensor_scalar(e1[:, j, :], s_cur[1][j][:],
                                            A8 * SCALE / (WS * WS), B8,
                                            Alu.mult, Alu.add)
                e_cur = [e0[:], e1[:].bitcast(f8)]
                s_next = ([emit_S(qt, 2 * p + 2), emit_S(qt, 2 * p + 3)]
                          if p + 1 < NP else None)
                # same-psum-bank matmuls back-to-back: [O,O] then [D,D]
                for i in range(2):
                    g = 2 * p + i
                    nc.tensor.matmul(po[:], vt_sb[:, 2 * g:2 * g + 2, :],
                                     e_cur[i],
                                     start=(g == 0), stop=(g == NGR - 1),
                                     perf_mode=DR)
                for i in range(2):
                    g = 2 * p + i
                    nc.tensor.matmul(pd[:], ones2[:, :, 0:1], e_cur[i],
                                     start=(g == 0), stop=(g == NGR - 1),
                                     perf_mode=DR)
                s_cur = s_next
            tails.append(emit_tail(qt, po, pd))
        tp, tms = tails.pop()
        tp()
        for f in tms:
            f()

    nc.compile()
    return nc


def _get_compiled():
    global _compiled_nc
    if _compiled_nc is None:
        _compiled_nc = _build()
    return _compiled_nc


def _ensure_ntff_hook():
    """Best-effort: register the axon NTFF profile hook so trace=True
    yields exec_time_ns. The image's antenv lacks axon_hooks; shim it."""
    import sys, types
    try:
        from antenv.axon_hooks import get_axon_ntff_profile_hook  # noqa: F401
        return
    except ImportError:
        pass
    try:
        mod = types.ModuleType("antenv.axon_hooks")
        _hook = [None]
        mod.set_axon_ntff_profile_hook = lambda h: _hook.__setitem__(0, h)
        mod.get_axon_ntff_profile_hook = lambda: _hook[0]
        sys.modules["antenv.axon_hooks"] = mod
        import antenv
        antenv.axon_hooks = mod
        from trn_agent_boot.trn_boot import _ntff_profile_via_ctypes
        mod.set_axon_ntff_profile_hook(
            _ntff_profile_via_ctypes("/opt/axon/libaxon_pjrt.so"))
    except Exception:
        pass


def kernel(x, Wq, bq, Wk, bk, Wv, bv, Wo, bo):
    global LAST_RESULT
    from concourse.bass_utils import run_bass_kernel_spmd

    nc = _get_compiled()
    f8 = ml_dtypes.float8_e4m3
    x = np.asarray(x, dtype=np.float32)
    b, c, d, h, w = x.shape
    n = d * h * w
    xf = x.reshape(b, c, n)
    Wq = np.asarray(Wq, np.float32)
    Wk = np.asarray(Wk, np.float32)
    Wv = np.asarray(Wv, np.float32)
    Wo = np.asarray(Wo, np.float32)
    bq = np.asarray(bq, np.float32)
    bk = np.asarray(bk, np.float32)
    bv = np.asarray(bv, np.float32)
    bo = np.asarray(bo, np.float32)

    def w3(Wm, gsl):
        # [128, NCC, GS] with w3[p, cc, m] = WS * Wm[g*GS+m, 128cc+p]
        a = (WS * Wm[gsl, :]).T            # [C, GS]
        return np.ascontiguousarray(
            a.reshape(NCC, 128, GS).transpose(1, 0, 2)).astype(f8)

    in_maps = []
    for core in range(8):
        bb, g = divmod(core, G)
        gsl = slice(g * GS, (g + 1) * GS)
        in_maps.append({
            "xb": np.ascontiguousarray(xf[bb]).astype(f8),
            "wq3": w3(Wq, gsl),
            "wk3": w3(Wk, gsl),
            "wv3": w3(Wv, gsl),
            "woT": np.ascontiguousarray(Wo[:, gsl].T / WS).astype(
                ml_dtypes.bfloat16),
            "bq": (WS * bq[gsl]).reshape(GS, 1).copy(),
            "bk": (WS * bk[gsl]).reshape(GS, 1).copy(),
            "bvb": np.ascontiguousarray(
                np.broadcast_to(WS * bv[gsl], (128, GS))).astype(np.float32),
        })

    trace = bool(os.environ.get("BASS_TRACE"))
    if trace:
        _ensure_ntff_hook()
    LAST_RESULT = run_bass_kernel_spmd(
        nc, in_maps, core_ids=list(range(8)), trace=trace)
    outs = LAST_RESULT.results

    out = np.empty((b, c, n), np.float32)
    for bb in range(b):
        acc = xf[bb] + bo[:, None]
        for g in range(G):
            o = outs[bb * G + g]
            den = o["denp"][0].astype(np.float32)
            acc = acc + o["outp"].astype(np.float32) / den[None, :]
        out[bb] = acc
    return out.reshape(b, c, d, h, w)
